# revision 11
# baseline (speedup 1.0000x reference)
"""Trainium2 Bass kernel for the GNN message-passing layer.

Strategy (pure data-parallel over batch, 8 NeuronCores, 16 batches/core):
  - Host pre-transposes activations to feature-major layout so the device
    does zero on-chip transposes: all compute runs in [feature, element]
    layout with 2 batches packed on the 128 SBUF partitions.
  - Edge update: the (M,K)-grid broadcast-add of the per-m / per-k node
    terms is folded into the TensorE pass as a second PSUM-accumulated
    matmul against a 0/1 indicator matrix, so VectorE never touches it.
  - leaky_relu runs on ScalarE (PSUM->SBUF, one pass).
  - The two mean-reductions run on VectorE as free-dim strided reduces.
  - Node updates are tiny block-diagonal matmuls (mean 1/64 folded into
    the host-prepared Wneigh weights).
"""
import sys

import numpy as np

sys.path.insert(0, "/opt/trn_rl_repo")

BS, M, K, FIN, FOUT, E = 128, 64, 64, 64, 64, 4096
NCORES = 8
BPC = BS // NCORES          # batches per core
PAIRS = BPC // 2            # 2 batches packed per 128 partitions
NEG_SLOPE = 0.01
WAVE = 1024                 # psum wave width (2 banks)
WAVES = E // WAVE

# The TensorE fp32 path runs at 4 cycles/row; float32r streams at 1 cycle/row
# for N>=256. Use f32r for the two big matmuls, plain fp32 for the small ones.
# fp32r operands must be *typed* fp32r end-to-end (walrus verifier requires the
# producing instruction to round), so the big-matmul operand tiles switch dtype.
USE_F32R = True

_CACHE = {}


def _build_nc(use_f32r=USE_F32R):
    import concourse.bacc as bacc
    import concourse.tile as tile
    from concourse import mybir
    from contextlib import ExitStack

    f32 = mybir.dt.float32
    f32r = mybir.dt.float32r
    fmm = f32r if use_f32r else f32
    Lrelu = mybir.ActivationFunctionType.Lrelu
    add = mybir.AluOpType.add
    X = mybir.AxisListType.X

    nc = bacc.Bacc("TRN2", target_bir_lowering=False, debug=False)

    # DRAM I/O (per core shard)
    xt_d = nc.dram_tensor("xt", [BPC, FIN, E], fmm, kind="ExternalInput").ap()
    zmt_d = nc.dram_tensor("zmt", [BPC, FIN, M], f32, kind="ExternalInput").ap()
    zkt_d = nc.dram_tensor("zkt", [BPC, FIN, K], f32, kind="ExternalInput").ap()
    wedge_d = nc.dram_tensor("wedge_bd", [128, 128], fmm, kind="ExternalInput").ap()
    wm_d = nc.dram_tensor("wm_bd", [128, 128], f32, kind="ExternalInput").ap()
    wk_d = nc.dram_tensor("wk_bd", [128, 128], f32, kind="ExternalInput").ap()
    wsm_d = nc.dram_tensor("wselfm_bd", [128, 128], f32, kind="ExternalInput").ap()
    wsk_d = nc.dram_tensor("wselfk_bd", [128, 128], f32, kind="ExternalInput").ap()
    wnm_d = nc.dram_tensor("wneighm_bd", [128, 128], f32, kind="ExternalInput").ap()
    wnk_d = nc.dram_tensor("wneighk_bd", [128, 128], f32, kind="ExternalInput").ap()
    eye_d = nc.dram_tensor("eye64", [64, 64], f32, kind="ExternalInput").ap()

    edges_o = nc.dram_tensor("edges_t", [BPC, FOUT, E], f32, kind="ExternalOutput").ap()
    zmup_o = nc.dram_tensor("zmup_t", [BPC, FOUT, M], f32, kind="ExternalOutput").ap()
    zkup_o = nc.dram_tensor("zkup_t", [BPC, FOUT, K], f32, kind="ExternalOutput").ap()

    with tile.TileContext(nc) as tc, ExitStack() as ctx:
        consts = ctx.enter_context(tc.tile_pool(name="consts", bufs=1))
        xts = ctx.enter_context(tc.tile_pool(name="xts", bufs=2))
        edges_pool = ctx.enter_context(tc.tile_pool(name="edges", bufs=2))
        small_in = ctx.enter_context(tc.tile_pool(name="small_in", bufs=2))
        work = ctx.enter_context(tc.tile_pool(name="work", bufs=2))
        psum_main = ctx.enter_context(tc.tile_pool(name="psmain", bufs=2, space="PSUM"))
        psum_small = ctx.enter_context(tc.tile_pool(name="pssmall", bufs=4, space="PSUM"))

        # ---- constants ----
        def load_const(name, ap_d, shape, dt=f32):
            t = consts.tile(shape, dt, tag=name)
            nc.sync.dma_start(out=t[:], in_=ap_d)
            return t

        wedge_sb = load_const("wedge", wedge_d, [128, 128], dt=fmm)
        wm_sb = load_const("wm", wm_d, [128, 128])
        wk_sb = load_const("wk", wk_d, [128, 128])
        wsm_sb = load_const("wsm", wsm_d, [128, 128])
        wsk_sb = load_const("wsk", wsk_d, [128, 128])
        wnm_sb = load_const("wnm", wnm_d, [128, 128])
        wnk_sb = load_const("wnk", wnk_d, [128, 128])
        eye_sb = load_const("eye", eye_d, [64, 64])

        # indicator [128, E]: rows 0-63 select m(i)=i//64, rows 64-127 select k(i)=i%64
        ind_sb = consts.tile([128, E], fmm, tag="ind")
        nc.scalar.copy(
            out=ind_sb[0:64, :].rearrange("p (m k) -> p m k", k=K),
            in_=eye_sb[:, :, None].to_broadcast([64, M, K]),
        )
        nc.scalar.copy(
            out=ind_sb[64:128, :].rearrange("p (m k) -> p m k", k=K),
            in_=eye_sb[:, None, :].to_broadcast([64, M, K]),
        )

        for p in range(PAIRS):
            b0 = 2 * p
            xt_pair = xts.tile([128, E], fmm, tag="xt_pair")
            nc.sync.dma_start(out=xt_pair[:], in_=xt_d[b0:b0 + 2].rearrange("b f e -> (b f) e"))
            zmt_pair = small_in.tile([128, M], f32, tag="zmt_pair")
            nc.sync.dma_start(out=zmt_pair[:], in_=zmt_d[b0:b0 + 2].rearrange("b f m -> (b f) m"))
            zkt_pair = small_in.tile([128, K], f32, tag="zkt_pair")
            nc.sync.dma_start(out=zkt_pair[:], in_=zkt_d[b0:b0 + 2].rearrange("b f k -> (b f) k"))

            # Wz_m / Wz_k in [node, (b, o)] layout -> stacked as lhsT for the S-matmul
            wzm_ps = psum_small.tile([64, 128], f32, tag="sm")
            nc.tensor.matmul(wzm_ps[:], zmt_pair[:], wm_sb[:], start=True, stop=True)
            wzk_ps = psum_small.tile([64, 128], f32, tag="sm")
            nc.tensor.matmul(wzk_ps[:], zkt_pair[:], wk_sb[:], start=True, stop=True)
            s_lhsT = work.tile([128, 128], fmm, tag="s_lhsT")
            nc.scalar.copy(out=s_lhsT[0:64, :], in_=wzm_ps[:])
            nc.scalar.copy(out=s_lhsT[64:128, :], in_=wzk_ps[:])

            edges_sb = edges_pool.tile([128, E], f32, tag="edges_sb")
            sum_nm = work.tile([128, M], f32, tag="sum_nm")
            sum_nk_parts = work.tile([128, WAVES, K], f32, tag="sum_nk_parts")
            sum_nk = work.tile([128, K], f32, tag="sum_nk")

            for w in range(WAVES):
                ps = psum_main.tile([128, WAVE], f32, tag="ps")
                for h in range(WAVE // 512):
                    lo = w * WAVE + h * 512
                    nc.tensor.matmul(
                        ps[:, h * 512:(h + 1) * 512],
                        s_lhsT[:],
                        ind_sb[:, lo:lo + 512],
                        start=True, stop=False,
                    )
                    nc.tensor.matmul(
                        ps[:, h * 512:(h + 1) * 512],
                        wedge_sb[:],
                        xt_pair[:, lo:lo + 512],
                        start=False, stop=True,
                    )
                wsl = slice(w * WAVE, (w + 1) * WAVE)
                nc.scalar.activation(
                    out=edges_sb[:, wsl], in_=ps[:], func=Lrelu, alpha=NEG_SLOPE,
                )
                mpw = WAVE // K  # m's per wave (16)
                nc.vector.tensor_reduce(
                    out=sum_nm[:, w * mpw:(w + 1) * mpw],
                    in_=edges_sb[:, wsl].rearrange("p (m k) -> p m k", k=K),
                    axis=X, op=add,
                )
                nc.vector.tensor_reduce(
                    out=sum_nk_parts[:, w, :],
                    in_=edges_sb[:, wsl].rearrange("p (m k) -> p k m", k=K),
                    axis=X, op=add,
                )

            nc.vector.tensor_reduce(
                out=sum_nk[:],
                in_=sum_nk_parts[:].rearrange("p w k -> p k w"),
                axis=X, op=add,
            )

            nc.sync.dma_start(
                out=edges_o[b0:b0 + 2].rearrange("b o e -> (b o) e"), in_=edges_sb[:]
            )

            # node updates
            node_m_ps = psum_small.tile([128, M], f32, tag="sm")
            nc.tensor.matmul(node_m_ps[:], wsm_sb[:], zmt_pair[:], start=True, stop=False)
            nc.tensor.matmul(node_m_ps[:], wnm_sb[:], sum_nm[:], start=False, stop=True)
            node_k_ps = psum_small.tile([128, K], f32, tag="sm")
            nc.tensor.matmul(node_k_ps[:], wsk_sb[:], zkt_pair[:], start=True, stop=False)
            nc.tensor.matmul(node_k_ps[:], wnk_sb[:], sum_nk[:], start=False, stop=True)

            node_m_sb = work.tile([128, M], f32, tag="node_m_sb")
            nc.scalar.activation(out=node_m_sb[:], in_=node_m_ps[:], func=Lrelu, alpha=NEG_SLOPE)
            node_k_sb = work.tile([128, K], f32, tag="node_k_sb")
            nc.scalar.activation(out=node_k_sb[:], in_=node_k_ps[:], func=Lrelu, alpha=NEG_SLOPE)

            nc.sync.dma_start(out=zmup_o[b0:b0 + 2].rearrange("b o m -> (b o) m"), in_=node_m_sb[:])
            nc.sync.dma_start(out=zkup_o[b0:b0 + 2].rearrange("b o k -> (b o) k"), in_=node_k_sb[:])

    nc.compile()
    return nc


def _blockdiag(w):
    out = np.zeros((128, 128), np.float32)
    out[:64, :64] = w
    out[64:, 64:] = w
    return out


def _host_prep(inputs):
    z_mk = np.ascontiguousarray(np.asarray(inputs["z_mk"], np.float32))
    z_m = np.asarray(inputs["z_m"], np.float32)
    z_k = np.asarray(inputs["z_k"], np.float32)
    xt = np.ascontiguousarray(z_mk.transpose(0, 2, 1))
    zmt = np.ascontiguousarray(z_m.transpose(0, 2, 1))
    zkt = np.ascontiguousarray(z_k.transpose(0, 2, 1))
    consts = {
        "wedge_bd": _blockdiag(np.asarray(inputs["Wedge"], np.float32).T),
        "wm_bd": _blockdiag(np.asarray(inputs["Wm"], np.float32).T),
        "wk_bd": _blockdiag(np.asarray(inputs["Wk"], np.float32).T),
        "wselfm_bd": _blockdiag(np.asarray(inputs["Wself_m"], np.float32).T),
        "wselfk_bd": _blockdiag(np.asarray(inputs["Wself_k"], np.float32).T),
        "wneighm_bd": _blockdiag(np.asarray(inputs["Wneigh_m"], np.float32).T / 64.0),
        "wneighk_bd": _blockdiag(np.asarray(inputs["Wneigh_k"], np.float32).T / 64.0),
        "eye64": np.eye(64, dtype=np.float32),
    }
    in_maps = []
    for c in range(NCORES):
        sl = slice(c * BPC, (c + 1) * BPC)
        m = {"xt": np.ascontiguousarray(xt[sl]),
             "zmt": np.ascontiguousarray(zmt[sl]),
             "zkt": np.ascontiguousarray(zkt[sl])}
        m.update(consts)
        in_maps.append(m)
    return in_maps


def _postprocess(results):
    edges_t = np.concatenate([r["edges_t"] for r in results], axis=0)
    zmup_t = np.concatenate([r["zmup_t"] for r in results], axis=0)
    zkup_t = np.concatenate([r["zkup_t"] for r in results], axis=0)
    z_mk_updated = np.ascontiguousarray(edges_t.transpose(0, 2, 1)).reshape(BS, M * K, FOUT)
    z_m_updated = np.ascontiguousarray(zmup_t.transpose(0, 2, 1))
    z_k_updated = np.ascontiguousarray(zkup_t.transpose(0, 2, 1))
    return z_mk_updated, z_m_updated, z_k_updated


def run_on_hw(in_maps, trace=False, **kwargs):
    from concourse.bass_utils import run_bass_kernel_spmd

    if "nc" not in _CACHE:
        _CACHE["nc"] = _build_nc()
    res = run_bass_kernel_spmd(
        _CACHE["nc"], in_maps, core_ids=list(range(NCORES)), trace=trace, **kwargs
    )
    return res


def kernel(**inputs):
    in_maps = _host_prep(inputs)
    res = run_on_hw(in_maps)
    return _postprocess(res.results)
